# revision 25
# baseline (speedup 1.0000x reference)
"""BFP-quantized 3x3 conv (nn_BFConv2d) on 8 Trainium2 NeuronCores.

Reference computation (see problem): bfp_quantize(x) with groups of 36
consecutive elements of the flattened tensor sharing an exponent (8 mantissa
bits), conv2d 3x3 pad 1, + bias, bfp_quantize(out).

Sharding: data-parallel over batch, 2 batches per core. BFP groups of the
flat (B,C,H,W) tensor do not align with batch boundaries (batch size mod 36
!= 0), so each core's flat range has a per-core phase p_k = (k*S) mod 36.
The kernel handles this exactly:
  - input slab per core starts at global flat (k*S - 36); the quantize pass
    starts at a runtime register offset o = (36 - p) % 36 so groups align
    with the GLOBAL 36-grid; quantized x (exactly representable in bf16) is
    written to a DRAM scratch with identical local indexing.
  - conv reads the scratch at static offset 36 (= local index of k*S).
  - conv also computes a small "head" row (last row of previous batch,
    channel C-1) and "tail" strip (first rows of next batch, channel 0) from
    host-prequantized halo strips, writing raw f32 conv+bias results to an
    extended scratch so that the core's OWNED aligned output range
    [R_k, R_{k+1}), R_k = 36*floor(k*S/36), is fully covered.
  - output quantize pass reads the raw scratch at runtime offset W - p
    (aligned to the global grid) and writes the final quantized output with
    static indexing; the host concatenates the per-core aligned ranges.
The only host-side fixup is the final (partial) group of the whole tensor,
recomputed from 8 raw values returned by core 7.

Quantization math: the whole pipeline runs in fp16 (tolerance is 2e-2;
fp16 keeps 11 mantissa bits vs the 8 the BFP format keeps, so the only
deviation from the f32 reference is rare double-rounding knife-edges).
For each group, C = 1.5 * 2**(e+3) where e = floor(log2(max|g|));
q = (x + C) - C in fp16 rounds x to the nearest multiple of 2**(e-7)
with round-half-to-even, identical to round(g/scale)*scale. C is built
from the group max via f32 exponent-field bit arithmetic, then
downconverted to fp16 (exact). Quantized values are exactly
representable in fp16, so the final output DMAs out as fp16 and the
host upcasts to f32 losslessly.
"""

from contextlib import ExitStack
from dataclasses import dataclass

import numpy as np

import concourse.bass as bass
import concourse.bacc as bacc
import concourse.mybir as mybir
import concourse.tile as tile

F32 = mybir.dt.float32
F16 = mybir.dt.float16
I32 = mybir.dt.int32
U32 = mybir.dt.uint32
U16 = mybir.dt.uint16
ALU = mybir.AluOpType

GSZ = 36
EXPMASK = 0x7F800000
MAGIC = 0x01C00000    # (3 << 23) | 0x400000 -> C = 1.5 * 2**(e+3) (f32 bits)
EXPMASK16 = 0x7C00
MAGIC16 = 0x0E00      # (3 << 10) | 0x200 -> C = 1.5 * 2**(e+3) (fp16 bits)


@dataclass(frozen=True)
class Cfg:
    B: int = 16          # total batches
    C: int = 32          # channels (in == out)
    H: int = 224
    W: int = 224
    ncores: int = 8
    R: int = 28          # conv row-block height (divides H, even)
    FT_A: int = 41       # groups per partition per quantize tile (input)
    NT_A: int = 17       # quantize tiles (input)
    FT_C: int = 41
    NT_C: int = 17
    TAILW: int = 72      # tail strip length (>= 71 guarantees coverage)

    @property
    def Z(self):
        return self.C * self.H * self.W

    @property
    def BPC(self):
        return self.B // self.ncores

    @property
    def S(self):
        return self.BPC * self.Z

    @property
    def NQ_A(self):
        return self.NT_A * 128 * self.FT_A

    @property
    def NQ_C(self):
        return self.NT_C * 128 * self.FT_C

    @property
    def LXA(self):
        return 36 + self.NQ_A * GSZ

    @property
    def XQ_LEN(self):
        return self.LXA

    @property
    def OUT_Q_LEN(self):
        return self.NQ_C * GSZ

    @property
    def OUT_EXT_LEN(self):
        return self.W + self.NQ_C * GSZ

    @property
    def TAILROWS(self):
        return -(-self.TAILW // self.W)

    def check(self):
        assert self.B % self.ncores == 0
        assert self.H % self.R == 0 and self.R % 4 == 0
        assert self.NQ_A * GSZ >= self.S + 71
        assert self.NQ_C * GSZ >= self.S + 71
        assert 2 * (self.W + 2) <= 512  # psum free-dim limit (f32)
        assert self.C == 32


CFG = Cfg()


def _phase(cfg, k):
    return (k * cfg.S) % GSZ


# --------------------------------------------------------------------------
# device kernel
# --------------------------------------------------------------------------

def _load_dyn(eng, dyn, col, lo, hi, nm):
    r = eng.alloc_register(nm)
    eng.reg_load(r, dyn[0:1, col:col + 1])
    return eng.snap(r, donate=True, min_val=lo, max_val=hi)


class _QuantPipe:
    """Software-pipelined group-of-36 BFP quantizer (fp16). stage1(i): load
    tile, group abs-max, magic constant bits (2 int16 ops), DMA-broadcast the
    constant to a full contiguous tile, broadcast add (gpsimd). stage2(i):
    contiguous subtract (vector), store. Emission defers stage2 by one tile
    so the vector engine never stalls on the gpsimd add."""

    LAG = 2

    def __init__(self, nc, pools, name, ft, src_ap_fn, dst_ap_fn, out_dt,
                 rd_eng, wr_eng):
        self.__dict__.update(locals())
        self.free = ft * GSZ
        self.pending = []

    def stage1(self, i):
        nc, name, free, ft = self.nc, self.name, self.free, self.ft
        pool, gpool = self.pools
        ta = pool.tile([128, free], F16, name=f"{name}_ta", tag="ta")
        self.rd_eng.dma_start(
            ta[:], self.src_ap_fn(i).rearrange("(p f) -> p f", p=128))
        gm = gpool.tile([128, ft], F16, name=f"{name}_gm", tag="gm")
        nc.vector.tensor_reduce(
            gm[:], ta[:].rearrange("p (g z) -> p g z", z=GSZ),
            axis=mybir.AxisListType.X, op=ALU.max, apply_absolute_value=True,
        )
        cb = gpool.tile([128, ft], U16, name=f"{name}_cb", tag="cb")
        nc.vector.tensor_scalar(
            cb[:], gm[:].bitcast(U16), scalar1=EXPMASK16, scalar2=None,
            op0=ALU.bitwise_and,
        )
        nc.vector.tensor_scalar(
            cb[:], cb[:], scalar1=MAGIC16, scalar2=None, op0=ALU.add,
        )
        cbc = cb[:].bitcast(F16).unsqueeze(-1).broadcast_to((128, ft, GSZ))
        tt = pool.tile([128, free], F16, name=f"{name}_tt", tag="tt")
        nc.gpsimd.tensor_add(
            tt[:].rearrange("p (g z) -> p g z", z=GSZ),
            ta[:].rearrange("p (g z) -> p g z", z=GSZ),
            cbc,
        )
        self.pending.append((i, tt, cbc))

    def stage2(self):
        nc, name, free = self.nc, self.name, self.free
        pool, _ = self.pools
        i, tt, cbc = self.pending.pop(0)
        tq = pool.tile([128, free], self.out_dt, name=f"{name}_tq", tag="tq")
        nc.vector.tensor_sub(
            tq[:].rearrange("p (g z) -> p g z", z=GSZ),
            tt[:].rearrange("p (g z) -> p g z", z=GSZ),
            cbc,
        )
        self.wr_eng.dma_start(
            self.dst_ap_fn(i).rearrange("(p f) -> p f", p=128), tq[:])

    def emit(self, i0, i1):
        for i in range(i0, i1):
            self.stage1(i)
            while len(self.pending) > self.LAG:
                self.stage2()

    def flush(self):
        while self.pending:
            self.stage2()


def _emit_shift_copies(nc, x96, L):
    """kw-shifted copies in partition groups 0/2 from group 1. Rows are
    pitched (W+2 wide with zero pad pairs between rows), so the shifts wrap
    through zeros and no per-block edge memsets are needed."""
    nc.sync.dma_start(x96[0:32, 1:L], x96[32:64, 0:L - 1])
    nc.scalar.dma_start(x96[64:96, 0:L - 1], x96[32:64, 1:L])


def build_nc(cfg: Cfg = CFG) -> bass.Bass:
    cfg.check()
    C, H, W, R = cfg.C, cfg.H, cfg.W, cfg.R
    Z, S = cfg.Z, cfg.S
    HW = H * W

    nc = bacc.Bacc("TRN2", target_bir_lowering=False, debug=False)

    xa = nc.dram_tensor("xa", [cfg.LXA], F16, kind="ExternalInput")
    xpre = nc.dram_tensor("xpre", [C, 2, W], F16, kind="ExternalInput")
    xpost = nc.dram_tensor("xpost", [C, cfg.TAILROWS + 1, W], F16,
                           kind="ExternalInput")
    wstk_in = nc.dram_tensor("wstk", [3, 96, C], F16, kind="ExternalInput")
    braw = nc.dram_tensor("braw", [C], F32, kind="ExternalInput")
    dyn = nc.dram_tensor("dyn", [1, 2], U32, kind="ExternalInput")

    out_q = nc.dram_tensor("out_q", [cfg.OUT_Q_LEN], F16, kind="ExternalOutput")
    rawtail = nc.dram_tensor("rawtail", [128], F16, kind="ExternalOutput")

    ctx = ExitStack()
    with tile.TileContext(nc) as tc:
        # ---- dynamic offsets: one register per engine that issues dynamic
        # DMAs (48 regs/engine, ~2 burned per dynamic DMA -> spread passes
        # over gpsimd / sync / scalar) ----
        off_o_gp = _load_dyn(nc.gpsimd, dyn, 0, 0, 35, "dyn_o_gp")
        off_o_sy = _load_dyn(nc.sync, dyn, 0, 0, 35, "dyn_o_sy")
        off_r_sc = _load_dyn(nc.scalar, dyn, 1, W - 35, W, "dyn_r_sc")

        xq_buf = nc.dram_tensor("xq_buf", [cfg.XQ_LEN], F16, kind="Internal")
        out_ext = nc.dram_tensor("out_ext", [cfg.OUT_EXT_LEN], F16,
                                 kind="Internal")

        # ---- stationary weights (host-prequantized, host-laid-out):
        # wstk[kh][g*32+c, co] = bfp_quantize(w)[co, c, kh, g] ----
        wpool = ctx.enter_context(tc.tile_pool(name="wpool", bufs=1))
        wstk = []
        for kh in range(3):
            wk = wpool.tile([96, C], F16, name=f"wstk{kh}")
            nc.sync.dma_start(wk[:], wstk_in[kh])
            wstk.append(wk)

        bias_sb = wpool.tile([C, 1], F32, name="bias_sb")
        nc.sync.dma_start(bias_sb[:], braw[:].rearrange("(c o) -> c o", o=1))
        bias64 = wpool.tile([64, 1], F32, name="bias64")
        nc.sync.dma_start(bias64[0:32, :], braw[:].rearrange("(c o) -> c o", o=1))
        nc.sync.dma_start(bias64[32:64, :], braw[:].rearrange("(c o) -> c o", o=1))

        # ---- quantize-pass chunking ----
        CH_A = 128 * cfg.FT_A * GSZ
        CH_C = 128 * cfg.FT_C * GSZ
        qa_pools = (ctx.enter_context(tc.tile_pool(name="qa_io", bufs=3)),
                    ctx.enter_context(tc.tile_pool(name="qa_g", bufs=4)))
        qc_pools = (ctx.enter_context(tc.tile_pool(name="qc_io", bufs=3)),
                    ctx.enter_context(tc.tile_pool(name="qc_g", bufs=4)))

        qa_pipe = _QuantPipe(
            nc, qa_pools, "qa", cfg.FT_A,
            lambda i: xa[bass.ds(off_o_gp + i * CH_A, CH_A)],
            lambda i: xq_buf[bass.ds(off_o_sy + i * CH_A, CH_A)],
            F16, rd_eng=nc.gpsimd, wr_eng=nc.sync)
        qc_pipe = _QuantPipe(
            nc, qc_pools, "qc", cfg.FT_C,
            lambda i: out_ext[bass.ds(off_r_sc + i * CH_C, CH_C)],
            lambda i: out_q[i * CH_C:(i + 1) * CH_C],
            F16, rd_eng=nc.scalar, wr_eng=nc.gpsimd)

        def emit_a(i0, i1):
            qa_pipe.emit(i0, i1)

        def emit_c(i0, i1):
            qc_pipe.emit(i0, i1)

        def a_hi(b):  # A tiles needed before conv of batch b can run
            return min(cfg.NT_A, -(-(36 + (b + 1) * Z) // CH_A))

        def c_hi(b):  # C tiles fully covered once conv batch b is done
            return min(cfg.NT_C, ((b + 1) * Z) // CH_C)

        # ---- conv machinery (pass B): conv + bias -> out_ext (f32, raw) ----
        xq3 = xq_buf[36:36 + S].rearrange("(b c hw) -> b c hw", b=cfg.BPC, c=C)
        oe3 = out_ext[W:W + S].rearrange("(b c hw) -> b c hw", b=cfg.BPC, c=C)

        xpool = ctx.enter_context(tc.tile_pool(name="xblk", bufs=4))
        opool = ctx.enter_context(tc.tile_pool(name="oblk", bufs=3))
        ppool = ctx.enter_context(tc.tile_pool(name="psum", bufs=8, space="PSUM"))

        PW = W + 2  # pitched row width (zero pad pair between rows)
        zrow = wpool.tile([32, PW], F16, name="zrow")
        nc.vector.memset(zrow[:], 0.0)

        def zero_pads(t, nrows):
            L = nrows * PW
            pads = t[32:64, W + 1:W + 1 + (nrows - 1) * PW].rearrange(
                "p (r u) -> p r u", u=PW)[:, :, 0:2]
            nc.vector.memset(pads, 0.0)
            nc.vector.memset(t[32:64, 0:1], 0.0)
            nc.vector.memset(t[32:64, L - 1:L], 0.0)

        def conv_quad(x96, ps, ra, rb):
            """One [64, 452] psum tile = two row-pairs in PE column-groups
            0/1. ra/rb = x96 row index of the kh=0 tap of each pair. Streams
            2W+2 pitched columns; psum cols W..W+1 (the pads) are discarded
            by the evict view."""
            for kh in range(3):
                c = (ra + kh) * PW + 1
                nc.tensor.matmul(
                    ps[0:32, 0:2 * W + 2], wstk[kh][:], x96[:, c:c + 2 * W + 2],
                    start=(kh == 0), stop=(kh == 2), tile_position=(0, 0),
                    skip_group_check=True,
                )
            for kh in range(3):
                c = (rb + kh) * PW + 1
                nc.tensor.matmul(
                    ps[32:64, 0:2 * W + 2], wstk[kh][:], x96[:, c:c + 2 * W + 2],
                    start=(kh == 0), stop=(kh == 2), tile_position=(0, 32),
                    skip_group_check=True,
                )

        def evict(dst, src):
            nc.scalar.activation(
                dst, src, mybir.ActivationFunctionType.Identity,
                bias=bias64[0:src.shape[0]])

        def ps_rows(ps, n):
            # [*, 2, W] view of a [*, 452] psum tile: rows at cols 0 and 226
            return ps[0:n, :].rearrange("p (t u) -> p t u", u=PW)[:, :, 0:W]

        def emit_conv_block(b, blk):
            h0 = blk * R
            lo = max(h0 - 1, 0)
            hi = min(h0 + R + 1, H)
            nrows = R + 2
            L = nrows * PW
            x96 = xpool.tile([96, L], F16, name="x96", tag="x96")
            g1r = x96[32:64, :].rearrange("p (r w) -> p r w", w=PW)
            r0 = lo - (h0 - 1)
            nc.sync.dma_start(
                g1r[:, r0:r0 + (hi - lo), 1:1 + W],
                xq3[b][:, lo * W:hi * W].rearrange("p (r w) -> p r w", w=W),
            )
            if h0 == 0:
                nc.sync.dma_start(x96[32:64, 0:PW], zrow[:])
            if hi == H:
                nc.sync.dma_start(x96[32:64, (nrows - 1) * PW:nrows * PW],
                                  zrow[:])
            _emit_shift_copies(nc, x96, L)
            # out_sb64: even row-pairs on partitions 0:32, odd on 32:64
            nq = R // 4              # quads per block
            out_sb = opool.tile([64, nq * 2 * W], F16, name="out_sb",
                                tag="out_sb")
            for q in range(nq):
                ps = ppool.tile([64, 2 * PW], F32, name="ps", tag="ps")
                conv_quad(x96, ps, 4 * q, 4 * q + 2)
                evict(out_sb[:, q * 2 * W:(q + 1) * 2 * W].rearrange(
                          "p (t w) -> p t w", w=W),
                      ps_rows(ps, 64))
            dst = oe3[b][:, h0 * W:(h0 + R) * W].rearrange(
                "c (q par f) -> c q par f", par=2, f=2 * W)
            nc.sync.dma_start(
                dst[:, :, 0, :],
                out_sb[0:32, :].rearrange("c (q f) -> c q f", f=2 * W))
            nc.sync.dma_start(
                dst[:, :, 1, :],
                out_sb[32:64, :].rearrange("c (q f) -> c q f", f=2 * W))

        hpool = ctx.enter_context(tc.tile_pool(name="hpool", bufs=1))

        def emit_head():
            # out(b=-1, c=C-1, h=H-1, :) -> out_ext[0:W]
            x96h = xpool.tile([96, 3 * PW], F16, name="x96h", tag="x96sp")
            nc.sync.dma_start(
                x96h[32:64, :].rearrange("p (r w) -> p r w", w=PW)[:, 0:2,
                                                                  1:1 + W],
                xpre[:])
            nc.sync.dma_start(x96h[32:64, 2 * PW:3 * PW], zrow[:])
            _emit_shift_copies(nc, x96h, 3 * PW)
            ps_h = ppool.tile([C, 2 * PW], F32, name="ps", tag="ps")
            for kh in range(3):
                c = kh * PW + 1
                nc.tensor.matmul(ps_h[:, 0:W], wstk[kh][:], x96h[:, c:c + W],
                                 start=(kh == 0), stop=(kh == 2))
            head_sb = hpool.tile([C, W], F16, name="head_sb")
            nc.scalar.activation(head_sb[:], ps_h[:, 0:W],
                                 mybir.ActivationFunctionType.Identity,
                                 bias=bias_sb[:])
            nc.sync.dma_start(out_ext[0:W].rearrange("(o w) -> o w", o=1),
                              head_sb[C - 1:C, :])

        def emit_tail():
            # out(b=BPC, c=0, h=0..TAILROWS-1, :) + zero gap fill
            trows = cfg.TAILROWS
            x96t = xpool.tile([96, (trows + 2) * PW], F16, name="x96t",
                              tag="x96sp")
            nc.sync.dma_start(x96t[32:64, 0:PW], zrow[:])
            nc.sync.dma_start(
                x96t[32:64, :].rearrange("p (r w) -> p r w",
                                         w=PW)[:, 1:trows + 2, 1:1 + W],
                xpost[:])
            _emit_shift_copies(nc, x96t, (trows + 2) * PW)
            tail_sb = hpool.tile([C, trows * W], F16, name="tail_sb")
            j = 0
            while j < trows:
                npair = 2 if j + 1 < trows else 1
                ps_t = ppool.tile([C, 2 * PW], F32, name="ps", tag="ps")
                if npair == 1:
                    c = j * PW + 1
                    for kh in range(3):
                        nc.tensor.matmul(
                            ps_t[:, 0:W], wstk[kh][:],
                            x96t[:, c + kh * PW:c + kh * PW + W],
                            start=(kh == 0), stop=(kh == 2))
                    nc.scalar.activation(tail_sb[:, j * W:(j + 1) * W],
                                         ps_t[:, 0:W],
                                         mybir.ActivationFunctionType.Identity,
                                         bias=bias_sb[:])
                else:
                    c = j * PW + 1
                    for kh in range(3):
                        nc.tensor.matmul(
                            ps_t[:, 0:2 * W + 2], wstk[kh][:],
                            x96t[:, c + kh * PW:c + kh * PW + 2 * W + 2],
                            start=(kh == 0), stop=(kh == 2))
                    nc.scalar.activation(
                        tail_sb[:, j * W:(j + 2) * W].rearrange(
                            "p (t w) -> p t w", w=W),
                        ps_rows(ps_t, C),
                        mybir.ActivationFunctionType.Identity,
                        bias=bias_sb[:])
                j += npair
            nc.sync.dma_start(
                out_ext[W + S:W + S + cfg.TAILW].rearrange("(o w) -> o w", o=1),
                tail_sb[0:1, 0:cfg.TAILW])
            gap_start = W + S + cfg.TAILW
            gap = cfg.OUT_EXT_LEN - gap_start
            assert 0 <= gap <= 16384, gap
            if gap:
                zt = hpool.tile([1, gap], F16, name="zt")
                nc.vector.memset(zt[:], 0.0)
                nc.sync.dma_start(
                    out_ext[gap_start:].rearrange("(o w) -> o w", o=1), zt[:])

        # ---- interleaved emission: quantize tiles spread between conv
        # blocks so the per-engine schedules alternate between passes ----
        a_done = [0]
        c_done = [0]

        def emit_a_upto(i1):
            if i1 > a_done[0]:
                emit_a(a_done[0], i1)
                a_done[0] = i1

        def emit_c_upto(i1):
            if i1 > c_done[0]:
                emit_c(c_done[0], i1)
                c_done[0] = i1

        nblk = H // R
        # pre-zero the pad columns of all ring buffers (once, off the
        # steady-state critical path), then head/tail strips (host data only)
        for _ in range(4):
            t = xpool.tile([96, (R + 2) * PW], F16, name="x96", tag="x96")
            zero_pads(t, R + 2)
        for _ in range(4):
            t = xpool.tile([96, 3 * PW], F16, name="x96sp", tag="x96sp")
            zero_pads(t, 3)
        emit_head()
        emit_tail()
        emit_a_upto(a_hi(0))
        qa_pipe.flush()
        for b in range(cfg.BPC):
            for blk in range(nblk):
                emit_conv_block(b, blk)
                # spread next batch's A tiles across this batch's blocks
                if b + 1 < cfg.BPC:
                    frac_a = a_hi(b) + (a_hi(b + 1) - a_hi(b)) * (blk + 1) // nblk
                    emit_a_upto(frac_a)
                    if blk == nblk - 1:
                        qa_pipe.flush()
                # spread C tiles of the previous batch across this batch
                if b > 0:
                    frac_c = c_hi(b - 2) if b >= 2 else 0
                    frac_c += (c_hi(b - 1) - frac_c) * (blk + 1) // nblk
                    emit_c_upto(frac_c)
        emit_c_upto(cfg.NT_C)
        qc_pipe.flush()

        # ---- rawtail: raw conv values around (k+1)S for host final-group fix
        rt_sb = hpool.tile([1, 128], F16, name="rt_sb")
        nc.sync.dma_start(
            rt_sb[:],
            out_ext[W + S - 56:W + S + 72].rearrange("(o w) -> o w", o=1))
        nc.sync.dma_start(rawtail[:].rearrange("(o w) -> o w", o=1), rt_sb[:])

        ctx.close()
    nc.compile()
    return nc


# --------------------------------------------------------------------------
# host side
# --------------------------------------------------------------------------

def host_bfp36(flat32):
    """f32 replica of the reference quantization (groups of 36)."""
    n = flat32.size
    pad = (-n) % GSZ
    g = np.concatenate([flat32, np.zeros(pad, np.float32)]).reshape(-1, GSZ)
    m = np.max(np.abs(g), axis=1)
    cbits = (m.view(np.uint32) & np.uint32(0x7F800000)) + np.uint32(0x08400000)
    Cc = cbits.view(np.float32)[:, None]
    q = (g + Cc) - Cc
    return q.reshape(-1)[:n]


def host_bfp36_f16(flat16):
    """Bit-exact replica of the DEVICE fp16 quantization (groups of 36)."""
    n = flat16.size
    pad = (-n) % GSZ
    g = np.concatenate([flat16, np.zeros(pad, np.float16)]).reshape(-1, GSZ)
    m32 = np.max(np.abs(g), axis=1).astype(np.float32)
    cbits = (m32.view(np.uint32) & np.uint32(EXPMASK)) + np.uint32(MAGIC)
    Cc = cbits.view(np.float32).astype(np.float16)[:, None]
    q = (g + Cc) - Cc
    return q.reshape(-1)[:n]


def shard_inputs(x, weight, bias, cfg: Cfg = CFG):
    B, C, H, W = cfg.B, cfg.C, cfg.H, cfg.W
    S, Z = cfg.S, cfg.Z
    xf = np.ascontiguousarray(x, dtype=np.float32).reshape(-1)
    x16 = xf.astype(np.float16)
    total = xf.size
    xq_full = host_bfp36_f16(x16).reshape(B, C, H, W)
    wq = host_bfp36(
        np.ascontiguousarray(weight, dtype=np.float32).reshape(-1)
    ).reshape(C, C, 3, 3)
    # wstk[kh, g*32+c, co] = wq[co, c, kh, g]
    wstk = np.ascontiguousarray(
        wq.transpose(2, 3, 1, 0).astype(np.float16))  # [kh, g, c, co]
    wstk = wstk.reshape(3, 3 * C, C)
    bf = np.ascontiguousarray(bias, dtype=np.float32)

    in_maps = []
    for k in range(cfg.ncores):
        p = _phase(cfg, k)
        start = k * S - 36
        xa = np.zeros(cfg.LXA, np.float16)
        s0, s1 = max(start, 0), min(start + cfg.LXA, total)
        xa[s0 - start:s1 - start] = x16[s0:s1]

        if k == 0:
            xpre = np.zeros((C, 2, W), np.float16)
        else:
            xpre = xq_full[2 * k - 1, :, H - 2:H, :]
        nxt = 2 * k + cfg.BPC
        if nxt >= B:
            xpost = np.zeros((C, cfg.TAILROWS + 1, W), np.float16)
        else:
            xpost = xq_full[nxt, :, 0:cfg.TAILROWS + 1, :]

        o = (36 - p) % 36
        r = W - p
        in_maps.append({
            "xa": xa,
            "xpre": np.ascontiguousarray(xpre),
            "xpost": np.ascontiguousarray(xpost),
            "wstk": wstk,
            "braw": bf,
            "dyn": np.array([[o, r]], dtype=np.uint32),
        })
    return in_maps


def unshard(results, cfg: Cfg = CFG):
    B, C, H, W = cfg.B, cfg.C, cfg.H, cfg.W
    S = cfg.S
    total = B * cfg.Z
    out = np.empty(total, np.float32)
    for k in range(cfg.ncores):
        Rk = k * S - _phase(cfg, k)
        Rk = max(Rk, 0)
        if k + 1 < cfg.ncores:
            Rn = (k + 1) * S - _phase(cfg, k + 1)
        else:
            Rn = total
        take = Rn - Rk
        out[Rk:Rn] = results[k]["out_q"][:take].astype(np.float32)
    # final partial group fixup from core 7 raw values
    gstart = (total // GSZ) * GSZ
    if gstart < total:
        nrem = total - gstart
        rt = results[cfg.ncores - 1]["rawtail"]
        # rawtail[j] = out_ext[W+S-56+j] = global ((k+1)S - 56 + j)
        j0 = gstart - (total - 56)
        raw = rt[j0:j0 + nrem].astype(np.float16)
        out[gstart:] = host_bfp36_f16(raw)[:nrem].astype(np.float32)
    return out.reshape(B, C, H, W)


_NC_CACHE = {}


def _get_nc(cfg: Cfg = CFG):
    if cfg not in _NC_CACHE:
        _NC_CACHE[cfg] = build_nc(cfg)
    return _NC_CACHE[cfg]


def kernel(x, weight, bias):
    from concourse.bass_utils import run_bass_kernel_spmd
    cfg = CFG
    nc = _get_nc(cfg)
    in_maps = shard_inputs(x, weight, bias, cfg)
    res = run_bass_kernel_spmd(nc, in_maps, core_ids=list(range(cfg.ncores)))
    return unshard(res.results, cfg)



# revision 26
# speedup vs baseline: 1.2758x; 1.2758x over previous
"""BFP-quantized 3x3 conv (nn_BFConv2d) on 8 Trainium2 NeuronCores.

Reference computation (see problem): bfp_quantize(x) with groups of 36
consecutive elements of the flattened tensor sharing an exponent (8 mantissa
bits), conv2d 3x3 pad 1, + bias, bfp_quantize(out).

Sharding: data-parallel over batch, 2 batches per core. BFP groups of the
flat (B,C,H,W) tensor do not align with batch boundaries (batch size mod 36
!= 0), so each core's flat range has a per-core phase p_k = (k*S) mod 36.
The kernel handles this exactly:
  - input slab per core starts at global flat (k*S - 36); the quantize pass
    starts at a runtime register offset o = (36 - p) % 36 so groups align
    with the GLOBAL 36-grid; quantized x (exactly representable in bf16) is
    written to a DRAM scratch with identical local indexing.
  - conv reads the scratch at static offset 36 (= local index of k*S).
  - conv also computes a small "head" row (last row of previous batch,
    channel C-1) and "tail" strip (first rows of next batch, channel 0) from
    host-prequantized halo strips, writing raw f32 conv+bias results to an
    extended scratch so that the core's OWNED aligned output range
    [R_k, R_{k+1}), R_k = 36*floor(k*S/36), is fully covered.
  - output quantize pass reads the raw scratch at runtime offset W - p
    (aligned to the global grid) and writes the final quantized output with
    static indexing; the host concatenates the per-core aligned ranges.
The only host-side fixup is the final (partial) group of the whole tensor,
recomputed from 8 raw values returned by core 7.

Quantization math: the whole pipeline runs in fp16 (tolerance is 2e-2;
fp16 keeps 11 mantissa bits vs the 8 the BFP format keeps, so the only
deviation from the f32 reference is rare double-rounding knife-edges).
For each group, C = 1.5 * 2**(e+3) where e = floor(log2(max|g|));
q = (x + C) - C in fp16 rounds x to the nearest multiple of 2**(e-7)
with round-half-to-even, identical to round(g/scale)*scale. C is built
from the group max via f32 exponent-field bit arithmetic, then
downconverted to fp16 (exact). Quantized values are exactly
representable in fp16, so the final output DMAs out as fp16 and the
host upcasts to f32 losslessly.
"""

from contextlib import ExitStack
from dataclasses import dataclass

import numpy as np

import concourse.bass as bass
import concourse.bacc as bacc
import concourse.mybir as mybir
import concourse.tile as tile

F32 = mybir.dt.float32
F16 = mybir.dt.float16
I32 = mybir.dt.int32
U32 = mybir.dt.uint32
U16 = mybir.dt.uint16
ALU = mybir.AluOpType

GSZ = 36
EXPMASK = 0x7F800000
MAGIC = 0x01C00000    # (3 << 23) | 0x400000 -> C = 1.5 * 2**(e+3) (f32 bits)
EXPMASK16 = 0x7C00
MAGIC16 = 0x0E00      # (3 << 10) | 0x200 -> C = 1.5 * 2**(e+3) (fp16 bits)


@dataclass(frozen=True)
class Cfg:
    B: int = 16          # total batches
    C: int = 32          # channels (in == out)
    H: int = 224
    W: int = 224
    ncores: int = 8
    R: int = 28          # conv row-block height (divides H, even)
    FT_A: int = 41       # groups per partition per quantize tile (input)
    NT_A: int = 17       # quantize tiles (input)
    FT_C: int = 41
    NT_C: int = 17
    TAILW: int = 72      # tail strip length (>= 71 guarantees coverage)

    @property
    def Z(self):
        return self.C * self.H * self.W

    @property
    def BPC(self):
        return self.B // self.ncores

    @property
    def S(self):
        return self.BPC * self.Z

    @property
    def NQ_A(self):
        return self.NT_A * 128 * self.FT_A

    @property
    def NQ_C(self):
        return self.NT_C * 128 * self.FT_C

    @property
    def LXA(self):
        return 36 + self.NQ_A * GSZ

    @property
    def XQ_LEN(self):
        return self.LXA

    @property
    def OUT_Q_LEN(self):
        return self.NQ_C * GSZ

    @property
    def OUT_EXT_LEN(self):
        return self.W + self.NQ_C * GSZ

    @property
    def TAILROWS(self):
        return -(-self.TAILW // self.W)

    def check(self):
        assert self.B % self.ncores == 0
        assert self.H % self.R == 0 and self.R % 4 == 0
        assert self.NQ_A * GSZ >= self.S + 71
        assert self.NQ_C * GSZ >= self.S + 71
        assert 2 * (self.W + 2) <= 512  # psum free-dim limit (f32)
        assert self.C == 32


CFG = Cfg()


def _phase(cfg, k):
    return (k * cfg.S) % GSZ


# --------------------------------------------------------------------------
# device kernel
# --------------------------------------------------------------------------

def _load_dyn(eng, dyn, col, lo, hi, nm):
    r = eng.alloc_register(nm)
    eng.reg_load(r, dyn[0:1, col:col + 1])
    return eng.snap(r, donate=True, min_val=lo, max_val=hi)


class _QuantPipe:
    """Software-pipelined group-of-36 BFP quantizer (fp16). stage1(i): load
    tile, group abs-max, magic constant bits (2 int16 ops), DMA-broadcast the
    constant to a full contiguous tile, broadcast add (gpsimd). stage2(i):
    contiguous subtract (vector), store. Emission defers stage2 by one tile
    so the vector engine never stalls on the gpsimd add."""

    LAG = 2

    def __init__(self, nc, pools, name, ft, src_ap_fn, dst_ap_fn, out_dt,
                 rd_eng, wr_eng):
        self.__dict__.update(locals())
        self.free = ft * GSZ
        self.pending = []

    def stage1(self, i):
        nc, name, free, ft = self.nc, self.name, self.free, self.ft
        pool, gpool = self.pools
        ta = pool.tile([128, free], F16, name=f"{name}_ta", tag="ta")
        self.rd_eng.dma_start(
            ta[:], self.src_ap_fn(i).rearrange("(p f) -> p f", p=128))
        gm = gpool.tile([128, ft], F16, name=f"{name}_gm", tag="gm")
        nc.vector.tensor_reduce(
            gm[:], ta[:].rearrange("p (g z) -> p g z", z=GSZ),
            axis=mybir.AxisListType.X, op=ALU.max, apply_absolute_value=True,
        )
        cb = gpool.tile([128, ft], U16, name=f"{name}_cb", tag="cb")
        nc.vector.tensor_scalar(
            cb[:], gm[:].bitcast(U16), scalar1=EXPMASK16, scalar2=None,
            op0=ALU.bitwise_and,
        )
        nc.vector.tensor_scalar(
            cb[:], cb[:], scalar1=MAGIC16, scalar2=None, op0=ALU.add,
        )
        cbc = cb[:].bitcast(F16).unsqueeze(-1).broadcast_to((128, ft, GSZ))
        tt = pool.tile([128, free], F16, name=f"{name}_tt", tag="tt")
        nc.gpsimd.tensor_add(
            tt[:].rearrange("p (g z) -> p g z", z=GSZ),
            ta[:].rearrange("p (g z) -> p g z", z=GSZ),
            cbc,
        )
        self.pending.append((i, tt, cbc))

    def stage2(self):
        nc, name, free = self.nc, self.name, self.free
        pool, _ = self.pools
        i, tt, cbc = self.pending.pop(0)
        tq = pool.tile([128, free], self.out_dt, name=f"{name}_tq", tag="tq")
        nc.vector.tensor_sub(
            tq[:].rearrange("p (g z) -> p g z", z=GSZ),
            tt[:].rearrange("p (g z) -> p g z", z=GSZ),
            cbc,
        )
        self.wr_eng.dma_start(
            self.dst_ap_fn(i).rearrange("(p f) -> p f", p=128), tq[:])

    def emit(self, i0, i1):
        for i in range(i0, i1):
            self.stage1(i)
            while len(self.pending) > self.LAG:
                self.stage2()

    def flush(self):
        while self.pending:
            self.stage2()


def _emit_shifted_copies(nc, x96, L, nrows, W):
    """Build kw-shifted copies in partition groups 0/2 from group 1 and zero
    the wrapped row-edge columns. The memsets run on gpsimd so the conv
    pipeline never waits on the vector engine (busy with quantize passes)."""
    nc.sync.dma_start(x96[0:32, 1:L], x96[32:64, 0:L - 1])
    nc.scalar.dma_start(x96[64:96, 0:L - 1], x96[32:64, 1:L])
    g0 = x96[0:32, :].rearrange("p (r w) -> p r w", w=W)
    nc.gpsimd.memset(g0[:, :, 0:1], 0.0)
    g2 = x96[64:96, :].rearrange("p (r w) -> p r w", w=W)
    nc.gpsimd.memset(g2[:, :, W - 1:W], 0.0)


def build_nc(cfg: Cfg = CFG) -> bass.Bass:
    cfg.check()
    C, H, W, R = cfg.C, cfg.H, cfg.W, cfg.R
    Z, S = cfg.Z, cfg.S
    HW = H * W

    nc = bacc.Bacc("TRN2", target_bir_lowering=False, debug=False)

    xa = nc.dram_tensor("xa", [cfg.LXA], F16, kind="ExternalInput")
    xpre = nc.dram_tensor("xpre", [C, 2, W], F16, kind="ExternalInput")
    xpost = nc.dram_tensor("xpost", [C, cfg.TAILROWS + 1, W], F16,
                           kind="ExternalInput")
    wstk_in = nc.dram_tensor("wstk", [3, 96, C], F16, kind="ExternalInput")
    braw = nc.dram_tensor("braw", [C], F32, kind="ExternalInput")
    dyn = nc.dram_tensor("dyn", [1, 2], U32, kind="ExternalInput")

    out_q = nc.dram_tensor("out_q", [cfg.OUT_Q_LEN], F16, kind="ExternalOutput")
    rawtail = nc.dram_tensor("rawtail", [128], F16, kind="ExternalOutput")

    ctx = ExitStack()
    with tile.TileContext(nc) as tc:
        # ---- dynamic offsets: one register per engine that issues dynamic
        # DMAs (48 regs/engine, ~2 burned per dynamic DMA -> spread passes
        # over gpsimd / sync / scalar) ----
        off_o_gp = _load_dyn(nc.gpsimd, dyn, 0, 0, 35, "dyn_o_gp")
        off_o_sy = _load_dyn(nc.sync, dyn, 0, 0, 35, "dyn_o_sy")
        off_r_sc = _load_dyn(nc.scalar, dyn, 1, W - 35, W, "dyn_r_sc")

        xq_buf = nc.dram_tensor("xq_buf", [cfg.XQ_LEN], F16, kind="Internal")
        out_ext = nc.dram_tensor("out_ext", [cfg.OUT_EXT_LEN], F16,
                                 kind="Internal")

        # ---- stationary weights (host-prequantized, host-laid-out):
        # wstk[kh][g*32+c, co] = bfp_quantize(w)[co, c, kh, g] ----
        wpool = ctx.enter_context(tc.tile_pool(name="wpool", bufs=1))
        wstk = []
        for kh in range(3):
            wk = wpool.tile([96, C], F16, name=f"wstk{kh}")
            nc.sync.dma_start(wk[:], wstk_in[kh])
            wstk.append(wk)

        bias_sb = wpool.tile([C, 1], F32, name="bias_sb")
        nc.sync.dma_start(bias_sb[:], braw[:].rearrange("(c o) -> c o", o=1))
        bias64 = wpool.tile([64, 1], F32, name="bias64")
        nc.sync.dma_start(bias64[0:32, :], braw[:].rearrange("(c o) -> c o", o=1))
        nc.sync.dma_start(bias64[32:64, :], braw[:].rearrange("(c o) -> c o", o=1))

        # ---- quantize-pass chunking ----
        CH_A = 128 * cfg.FT_A * GSZ
        CH_C = 128 * cfg.FT_C * GSZ
        qa_pools = (ctx.enter_context(tc.tile_pool(name="qa_io", bufs=3)),
                    ctx.enter_context(tc.tile_pool(name="qa_g", bufs=4)))
        qc_pools = (ctx.enter_context(tc.tile_pool(name="qc_io", bufs=3)),
                    ctx.enter_context(tc.tile_pool(name="qc_g", bufs=4)))

        qa_pipe = _QuantPipe(
            nc, qa_pools, "qa", cfg.FT_A,
            lambda i: xa[bass.ds(off_o_gp + i * CH_A, CH_A)],
            lambda i: xq_buf[bass.ds(off_o_sy + i * CH_A, CH_A)],
            F16, rd_eng=nc.gpsimd, wr_eng=nc.sync)
        qc_pipe = _QuantPipe(
            nc, qc_pools, "qc", cfg.FT_C,
            lambda i: out_ext[bass.ds(off_r_sc + i * CH_C, CH_C)],
            lambda i: out_q[i * CH_C:(i + 1) * CH_C],
            F16, rd_eng=nc.scalar, wr_eng=nc.gpsimd)

        def emit_a(i0, i1):
            qa_pipe.emit(i0, i1)

        def emit_c(i0, i1):
            qc_pipe.emit(i0, i1)

        def a_hi(b):  # A tiles needed before conv of batch b can run
            return min(cfg.NT_A, -(-(36 + (b + 1) * Z) // CH_A))

        def c_hi(b):  # C tiles fully covered once conv batch b is done
            return min(cfg.NT_C, ((b + 1) * Z) // CH_C)

        # ---- conv machinery (pass B): conv + bias -> out_ext (f32, raw) ----
        xq3 = xq_buf[36:36 + S].rearrange("(b c hw) -> b c hw", b=cfg.BPC, c=C)
        oe3 = out_ext[W:W + S].rearrange("(b c hw) -> b c hw", b=cfg.BPC, c=C)

        xpool = ctx.enter_context(tc.tile_pool(name="xblk", bufs=4))
        opool = ctx.enter_context(tc.tile_pool(name="oblk", bufs=3))
        ppool = ctx.enter_context(tc.tile_pool(name="psum", bufs=8, space="PSUM"))

        def conv_quad(x96, ps, c0, c1):
            """One [64, 2W] psum tile = two row-pairs computed in PE column-
            groups 0/1. c0/c1 = x96 column bases of the kh=0 tap of each."""
            for kh in range(3):
                nc.tensor.matmul(
                    ps[0:32, :], wstk[kh][:],
                    x96[:, c0 + kh * W:c0 + kh * W + 2 * W],
                    start=(kh == 0), stop=(kh == 2), tile_position=(0, 0),
                    skip_group_check=True,
                )
            for kh in range(3):
                nc.tensor.matmul(
                    ps[32:64, :], wstk[kh][:],
                    x96[:, c1 + kh * W:c1 + kh * W + 2 * W],
                    start=(kh == 0), stop=(kh == 2), tile_position=(0, 32),
                    skip_group_check=True,
                )

        def evict(dst, src):
            nc.scalar.activation(
                dst, src, mybir.ActivationFunctionType.Identity,
                bias=bias64[0:src.shape[0]])

        def emit_conv_block(b, blk):
            h0 = blk * R
            lo = max(h0 - 1, 0)
            hi = min(h0 + R + 1, H)
            nrows = R + 2
            x96 = xpool.tile([96, nrows * W], F16, name="x96", tag="x96")
            if h0 == 0:
                nc.gpsimd.memset(x96[32:64, 0:W], 0.0)
            if hi == H:
                nc.gpsimd.memset(x96[32:64, (nrows - 1) * W:nrows * W], 0.0)
            dst_lo = (lo - (h0 - 1)) * W
            nc.sync.dma_start(
                x96[32:64, dst_lo:dst_lo + (hi - lo) * W],
                xq3[b][:, lo * W:hi * W],
            )
            _emit_shifted_copies(nc, x96, nrows * W, nrows, W)
            # out_sb64: even row-pairs on partitions 0:32, odd on 32:64
            nq = R // 4              # quads per block
            out_sb = opool.tile([64, nq * 2 * W], F16, name="out_sb",
                                tag="out_sb")
            for q in range(nq):
                ps = ppool.tile([64, 2 * W], F32, name="ps", tag="ps")
                conv_quad(x96, ps, (4 * q) * W, (4 * q + 2) * W)
                evict(out_sb[:, q * 2 * W:(q + 1) * 2 * W], ps[:])
            dst = oe3[b][:, h0 * W:(h0 + R) * W].rearrange(
                "c (q par f) -> c q par f", par=2, f=2 * W)
            nc.sync.dma_start(
                dst[:, :, 0, :],
                out_sb[0:32, :].rearrange("c (q f) -> c q f", f=2 * W))
            nc.sync.dma_start(
                dst[:, :, 1, :],
                out_sb[32:64, :].rearrange("c (q f) -> c q f", f=2 * W))

        hpool = ctx.enter_context(tc.tile_pool(name="hpool", bufs=1))

        def emit_head():
            # out(b=-1, c=C-1, h=H-1, :) -> out_ext[0:W]
            x96h = xpool.tile([96, 3 * W], F16, name="x96h", tag="x96sp")
            nc.sync.dma_start(
                x96h[32:64, 0:2 * W], xpre[:].rearrange("c r w -> c (r w)"))
            nc.gpsimd.memset(x96h[32:64, 2 * W:3 * W], 0.0)
            _emit_shifted_copies(nc, x96h, 3 * W, 3, W)
            ps_h = ppool.tile([C, 2 * W], F32, name="ps", tag="ps")
            for kh in range(3):
                nc.tensor.matmul(ps_h[:, 0:W], wstk[kh][:],
                                 x96h[:, kh * W:(kh + 1) * W],
                                 start=(kh == 0), stop=(kh == 2))
            head_sb = hpool.tile([C, W], F16, name="head_sb")
            nc.scalar.activation(head_sb[:], ps_h[:, 0:W],
                                 mybir.ActivationFunctionType.Identity,
                                 bias=bias_sb[:])
            nc.sync.dma_start(out_ext[0:W].rearrange("(o w) -> o w", o=1),
                              head_sb[C - 1:C, :])

        def emit_tail():
            # out(b=BPC, c=0, h=0..TAILROWS-1, :) + zero gap fill
            trows = cfg.TAILROWS
            x96t = xpool.tile([96, (trows + 2) * W], F16, name="x96t",
                              tag="x96sp")
            nc.gpsimd.memset(x96t[32:64, 0:W], 0.0)
            nc.sync.dma_start(
                x96t[32:64, W:(trows + 2) * W],
                xpost[:].rearrange("c r w -> c (r w)"))
            _emit_shifted_copies(nc, x96t, (trows + 2) * W, trows + 2, W)
            tail_sb = hpool.tile([C, trows * W], F16, name="tail_sb")
            j = 0
            while j < trows:
                npair = 2 if j + 1 < trows else 1
                n = npair * W
                ps_t = ppool.tile([C, 2 * W], F32, name="ps", tag="ps")
                for kh in range(3):
                    nc.tensor.matmul(ps_t[:, 0:n], wstk[kh][:],
                                     x96t[:, (j + kh) * W:(j + kh) * W + n],
                                     start=(kh == 0), stop=(kh == 2))
                nc.scalar.activation(tail_sb[:, j * W:j * W + n], ps_t[:, 0:n],
                                     mybir.ActivationFunctionType.Identity,
                                     bias=bias_sb[:])
                j += npair
            nc.sync.dma_start(
                out_ext[W + S:W + S + cfg.TAILW].rearrange("(o w) -> o w", o=1),
                tail_sb[0:1, 0:cfg.TAILW])
            gap_start = W + S + cfg.TAILW
            gap = cfg.OUT_EXT_LEN - gap_start
            assert 0 <= gap <= 16384, gap
            if gap:
                zt = hpool.tile([1, gap], F16, name="zt")
                nc.vector.memset(zt[:], 0.0)
                nc.sync.dma_start(
                    out_ext[gap_start:].rearrange("(o w) -> o w", o=1), zt[:])

        # ---- interleaved emission: quantize tiles spread between conv
        # blocks so the per-engine schedules alternate between passes ----
        a_done = [0]
        c_done = [0]

        def emit_a_upto(i1):
            if i1 > a_done[0]:
                emit_a(a_done[0], i1)
                a_done[0] = i1

        def emit_c_upto(i1):
            if i1 > c_done[0]:
                emit_c(c_done[0], i1)
                c_done[0] = i1

        nblk = H // R
        # head/tail strips depend only on host inputs: emit first so the
        # tensor/scalar engines have work while pass A warms up
        emit_head()
        emit_tail()
        emit_a_upto(a_hi(0))
        qa_pipe.flush()
        for b in range(cfg.BPC):
            for blk in range(nblk):
                emit_conv_block(b, blk)
                # spread next batch's A tiles across this batch's blocks
                if b + 1 < cfg.BPC:
                    frac_a = a_hi(b) + (a_hi(b + 1) - a_hi(b)) * (blk + 1) // nblk
                    emit_a_upto(frac_a)
                    if blk == nblk - 1:
                        qa_pipe.flush()
                # spread C tiles of the previous batch across this batch
                if b > 0:
                    frac_c = c_hi(b - 2) if b >= 2 else 0
                    frac_c += (c_hi(b - 1) - frac_c) * (blk + 1) // nblk
                    emit_c_upto(frac_c)
        emit_c_upto(cfg.NT_C)
        qc_pipe.flush()

        # ---- rawtail: raw conv values around (k+1)S for host final-group fix
        rt_sb = hpool.tile([1, 128], F16, name="rt_sb")
        nc.sync.dma_start(
            rt_sb[:],
            out_ext[W + S - 56:W + S + 72].rearrange("(o w) -> o w", o=1))
        nc.sync.dma_start(rawtail[:].rearrange("(o w) -> o w", o=1), rt_sb[:])

        ctx.close()
    nc.compile()
    return nc


# --------------------------------------------------------------------------
# host side
# --------------------------------------------------------------------------

def host_bfp36(flat32):
    """f32 replica of the reference quantization (groups of 36)."""
    n = flat32.size
    pad = (-n) % GSZ
    g = np.concatenate([flat32, np.zeros(pad, np.float32)]).reshape(-1, GSZ)
    m = np.max(np.abs(g), axis=1)
    cbits = (m.view(np.uint32) & np.uint32(0x7F800000)) + np.uint32(0x08400000)
    Cc = cbits.view(np.float32)[:, None]
    q = (g + Cc) - Cc
    return q.reshape(-1)[:n]


def host_bfp36_f16(flat16):
    """Bit-exact replica of the DEVICE fp16 quantization (groups of 36)."""
    n = flat16.size
    pad = (-n) % GSZ
    g = np.concatenate([flat16, np.zeros(pad, np.float16)]).reshape(-1, GSZ)
    m32 = np.max(np.abs(g), axis=1).astype(np.float32)
    cbits = (m32.view(np.uint32) & np.uint32(EXPMASK)) + np.uint32(MAGIC)
    Cc = cbits.view(np.float32).astype(np.float16)[:, None]
    q = (g + Cc) - Cc
    return q.reshape(-1)[:n]


def shard_inputs(x, weight, bias, cfg: Cfg = CFG):
    B, C, H, W = cfg.B, cfg.C, cfg.H, cfg.W
    S, Z = cfg.S, cfg.Z
    xf = np.ascontiguousarray(x, dtype=np.float32).reshape(-1)
    x16 = xf.astype(np.float16)
    total = xf.size
    xq_full = host_bfp36_f16(x16).reshape(B, C, H, W)
    wq = host_bfp36(
        np.ascontiguousarray(weight, dtype=np.float32).reshape(-1)
    ).reshape(C, C, 3, 3)
    # wstk[kh, g*32+c, co] = wq[co, c, kh, g]
    wstk = np.ascontiguousarray(
        wq.transpose(2, 3, 1, 0).astype(np.float16))  # [kh, g, c, co]
    wstk = wstk.reshape(3, 3 * C, C)
    bf = np.ascontiguousarray(bias, dtype=np.float32)

    in_maps = []
    for k in range(cfg.ncores):
        p = _phase(cfg, k)
        start = k * S - 36
        xa = np.zeros(cfg.LXA, np.float16)
        s0, s1 = max(start, 0), min(start + cfg.LXA, total)
        xa[s0 - start:s1 - start] = x16[s0:s1]

        if k == 0:
            xpre = np.zeros((C, 2, W), np.float16)
        else:
            xpre = xq_full[2 * k - 1, :, H - 2:H, :]
        nxt = 2 * k + cfg.BPC
        if nxt >= B:
            xpost = np.zeros((C, cfg.TAILROWS + 1, W), np.float16)
        else:
            xpost = xq_full[nxt, :, 0:cfg.TAILROWS + 1, :]

        o = (36 - p) % 36
        r = W - p
        in_maps.append({
            "xa": xa,
            "xpre": np.ascontiguousarray(xpre),
            "xpost": np.ascontiguousarray(xpost),
            "wstk": wstk,
            "braw": bf,
            "dyn": np.array([[o, r]], dtype=np.uint32),
        })
    return in_maps


def unshard(results, cfg: Cfg = CFG):
    B, C, H, W = cfg.B, cfg.C, cfg.H, cfg.W
    S = cfg.S
    total = B * cfg.Z
    out = np.empty(total, np.float32)
    for k in range(cfg.ncores):
        Rk = k * S - _phase(cfg, k)
        Rk = max(Rk, 0)
        if k + 1 < cfg.ncores:
            Rn = (k + 1) * S - _phase(cfg, k + 1)
        else:
            Rn = total
        take = Rn - Rk
        out[Rk:Rn] = results[k]["out_q"][:take].astype(np.float32)
    # final partial group fixup from core 7 raw values
    gstart = (total // GSZ) * GSZ
    if gstart < total:
        nrem = total - gstart
        rt = results[cfg.ncores - 1]["rawtail"]
        # rawtail[j] = out_ext[W+S-56+j] = global ((k+1)S - 56 + j)
        j0 = gstart - (total - 56)
        raw = rt[j0:j0 + nrem].astype(np.float16)
        out[gstart:] = host_bfp36_f16(raw)[:nrem].astype(np.float32)
    return out.reshape(B, C, H, W)


_NC_CACHE = {}


def _get_nc(cfg: Cfg = CFG):
    if cfg not in _NC_CACHE:
        _NC_CACHE[cfg] = build_nc(cfg)
    return _NC_CACHE[cfg]


def kernel(x, weight, bias):
    from concourse.bass_utils import run_bass_kernel_spmd
    cfg = CFG
    nc = _get_nc(cfg)
    in_maps = shard_inputs(x, weight, bias, cfg)
    res = run_bass_kernel_spmd(nc, in_maps, core_ids=list(range(cfg.ncores)))
    return unshard(res.results, cfg)

